# revision 1
# baseline (speedup 1.0000x reference)
"""Trainium2 Bass kernel for the nn_Aggregate GNN message-passing problem.

Computation (see reference):
    keep = (A > 0) limited to the first `neibor_num` set entries per row
    nb_mean = (keep @ X) / max(cnt, 1)
    out = leaky_relu(X @ W_line.T + b_line)
        + where(cnt > 0, leaky_relu(nb_mean @ W_nb.T + b_nb), 0)

Sharding: rows of A / output rows are split across 8 cores (1024 rows each).
No collectives are needed: each core gets its A row-block (transposed), its
X row-block (transposed), the shared X head rows, and the weights.

Key structural fact exploited: `keep` zeroes every set bit after the
`neibor_num`-th, so only the first C columns of A can contribute, where C
bounds the column position of the nn-th set bit over all rows.  The host
verifies exactly (cheaply) that every row reaches `neibor_num` set bits
within the first C=256 columns; in that case cnt == neibor_num for every
row and the kernel contracts over 256 neighbor candidates instead of 8192.
If the check fails (it cannot for the target input distribution), a numpy
fallback computes the exact reference semantics.

Device pipeline per core (rows R=1024, C=256, Cin=Cout=512):
  1. mask:    the host ships mbT[j, r] = (A[r, j] > 0) directly as fp8
              0/1 in transposed layout (exact, 1 byte/entry, and a valid
              PE operand dtype - no on-device convert needed).
  2. prefix:  cumT = LTRI.T @ mbT per 128-column chunk (+ ONES.T @ mbT of
              earlier chunks) gives the inclusive prefix count of set bits
              along the row, in transposed layout, on the PE (fp8 inputs,
              fp32 accumulation; counts <= 256 so exact).
  3. keepT = (cumT <= nn) * mbT                      (one fused DVE op)
  4. Xw = X_head @ (W_nb.T/nn) + 1 (x) (b_nb/nn)     (PE; no mask dep)
  5. xj = leaky(keep @ Xw)                           (PE + ACT Lrelu)
     xi = leaky(X_blk @ W_line.T + b_line)           (PE + ACT Lrelu)
     out = xi + xj                                   (DVE, fp32)
Step 4 uses associativity ((keep @ Xh) @ W == keep @ (Xh @ W)) to shrink
the neighbor stage from 48 matmuls to 24, and it depends only on weights,
so it fills the PE while the mask pipeline resolves.  Since cnt == nn on
the fast path, adding b_nb/nn to every row of Xw makes keep @ Xw land
exactly +b_nb - the per-tile xj bias matmuls collapse into 2 rank-1 terms.
The xj stage is emitted before the xi stage: the in-order PE queue must
not block ready xj work behind xi matmuls that wait on the delayed
xt/wlt bulk DMA.  xi biases ride k=1 ones-row matmuls in the PSUM group.

Precision: all matmuls accumulate fp32 in PSUM.  The mask/count side
(A-mask, LTRI/ONES) is exact fp8; the keep-mask and neighbor-mean path
(X_head, nbm, W_nb) is bf16 - masks are integer-exact and the
neighbor-mean path has small magnitudes, so its bf16 rounding
contributes little.  The precision-critical self-linear (X @ W_line.T,
operands at full scale) runs in fp16 (e5m10).  Measured absmax error vs
the fp32 reference: ~4e-4 of the output scale.

DMA strategy: each logical input is packed on the host into one wide
[128, *] (or [1, *]) tensor; the A-block load (which gates the whole PE
schedule) is split 4 ways across two SW-DGE queues so it lands on
parallel DMA engines, and the bulk stage-2 operands are held behind it
with explicit dependencies so they don't steal HBM bandwidth from the
critical path.
"""

import numpy as np

NCORES = 8
N = 8192
CIN = 512
COUT = 512
R = N // NCORES          # rows per core
C = 256                  # neighbor-candidate column window
KC = C // 128            # 128-col chunks of the window
MC = CIN // 128          # 128-row chunks of the feature dim
RT = R // 128            # 128-row output tiles per core
NEG_SLOPE = 0.01         # jax.nn.leaky_relu default

_nc_cache = {}
LAST_RESULT = None       # BassKernelResults of the most recent device run
SIM_SAFE = False         # CoreSim lacks Lrelu; True swaps in a Relu decomposition


def _build_nc(nn: int):
    import concourse.bass as bass
    import concourse.bacc as bacc
    import concourse.mybir as mybir
    import concourse.tile as tile
    from concourse.tile import add_dep_helper

    F32 = mybir.dt.float32
    BF16 = mybir.dt.bfloat16  # PE fast path for the mask/xj side
    FP16 = mybir.dt.float16   # e5m10 for the precision-critical xi path
    FP8 = mybir.dt.float8e4   # exact for the 0/1 mask; skips any convert op
    AF = mybir.ActivationFunctionType
    OP = mybir.AluOpType

    nc = bacc.Bacc("TRN2", target_bir_lowering=False, debug=False)

    at_d = nc.dram_tensor("at", [128, KC * R], FP8, kind="ExternalInput")
    xht_d = nc.dram_tensor("xht", [128, MC * C], BF16, kind="ExternalInput")
    xt_d = nc.dram_tensor("xt", [128, MC * R], FP16, kind="ExternalInput")
    wnbt_d = nc.dram_tensor("wnbt", [128, MC * COUT], BF16, kind="ExternalInput")
    wlt_d = nc.dram_tensor("wlt", [128, MC * COUT], FP16, kind="ExternalInput")
    sm_d = nc.dram_tensor("sm", [128, 256], FP8, kind="ExternalInput")
    rcb_d = nc.dram_tensor("rcb", [1, COUT + 128], BF16, kind="ExternalInput")
    rcf_d = nc.dram_tensor("rcf", [1, COUT + 128], FP16, kind="ExternalInput")
    out_d = nc.dram_tensor("out", [R, COUT], F32, kind="ExternalOutput")

    with tile.TileContext(nc) as tc:
        with (
            tc.tile_pool(name="const", bufs=1) as constp,
            tc.tile_pool(name="mask", bufs=1) as maskp,
            tc.tile_pool(name="work", bufs=3) as workp,
            tc.tile_pool(name="xjbuf", bufs=8) as xjp,
            tc.tile_pool(name="psum2", bufs=2, space=bass.MemorySpace.PSUM) as psump2,
            tc.tile_pool(name="psum3", bufs=3, space=bass.MemorySpace.PSUM) as psump3,
        ):
            # --- latency-critical loads (SW DGE: aggregates the 2KB lines
            # of these small tensors into 16KB packets) ------------------
            at_sb = maskp.tile([128, KC * R], FP8, name="at_sb")
            at_dmas = [(nc.gpsimd if p % 2 == 0 else nc.scalar).dma_start(
                           at_sb[p * 32:(p + 1) * 32, :],
                           at_d[p * 32:(p + 1) * 32, :])
                       for p in range(4)]
            sm = constp.tile([128, 256], FP8, name="sm_sb")
            nc.scalar.dma_start(sm[:], sm_d[:])
            xht_sb = constp.tile([128, MC * C], BF16, name="xht_sb")
            xh_half = MC * C // 2
            nc.scalar.dma_start(xht_sb[:, :xh_half], xht_d[:, :xh_half])
            nc.scalar.dma_start(xht_sb[:, xh_half:], xht_d[:, xh_half:])
            wnbt_sb = constp.tile([128, MC * COUT], BF16, name="wnbt_sb")
            wh = MC * COUT // 2
            nc.gpsimd.dma_start(wnbt_sb[:, :wh], wnbt_d[:, :wh])
            nc.gpsimd.dma_start(wnbt_sb[:, wh:], wnbt_d[:, wh:])
            rcb = constp.tile([1, COUT + 128], BF16, name="rcb_sb")
            nc.scalar.dma_start(rcb[:], rcb_d[:])
            rcf = constp.tile([1, COUT + 128], FP16, name="rcf_sb")
            nc.scalar.dma_start(rcf[:], rcf_d[:])

            ltri = sm[:, 0:128]
            ones = sm[:, 128:256]
            bnbs = rcb[:, 0:COUT]      # b_nb / nn (folded into Xw)
            onesb = rcb[:, COUT:]
            bl = rcf[:, 0:COUT]
            onesf = rcf[:, COUT:]
            at = [at_sb[:, t * R:(t + 1) * R] for t in range(KC)]
            wnbt = [wnbt_sb[:, m * COUT:(m + 1) * COUT] for m in range(MC)]

            # --- bulk stage-2 operands (HW DGE; 4-8KB lines).  Held back
            # behind the at transfer so the mask pipeline (which gates the
            # whole PE schedule) isn't starved of HBM bandwidth. ----------
            wlt_sb = constp.tile([128, MC * COUT], FP16, name="wlt_sb")
            d2 = nc.sync.dma_start(wlt_sb[:], wlt_d[:])
            xt_sb = constp.tile([128, MC * R], FP16, name="xt_sb")
            c1, c2 = 2 * CIN, 4 * CIN
            d1a = nc.sync.dma_start(xt_sb[:, :c1], xt_d[:, :c1])
            d1b = nc.sync.dma_start(xt_sb[:, c1:c2], xt_d[:, c1:c2])
            d1c = nc.sync.dma_start(xt_sb[:, c2:], xt_d[:, c2:])
            for b in (d2, d1a, d1b, d1c):
                for a in at_dmas:
                    add_dep_helper(b.ins, a.ins, sync=True,
                                   reason="bulk loads yield HBM to the mask path")
            # xt is packed per r-tile: block (r, m) at r*CIN + m*128
            wlt = [wlt_sb[:, m * COUT:(m + 1) * COUT] for m in range(MC)]

            # 1. the host ships (A > 0) directly as exact fp8 0/1
            mb = at

            # 2+3. prefix count along the row (PE) -> keep mask (DVE)
            keep = []
            for t in range(KC):
                keep_t = maskp.tile([128, R], BF16, name=f"keep{t}")
                keep.append(keep_t)
            # h-outer: both chunks' first row-halves (which gate the first
            # xj tiles) come off the DVE before the second halves
            for h in range(R // 512):
                for t in range(KC):
                    sl = slice(h * 512, (h + 1) * 512)
                    cum = psump3.tile([128, 512], F32, name="cum")
                    for s in range(t + 1):
                        nc.tensor.matmul(
                            cum[:],
                            ltri if s == t else ones,
                            mb[s][:, sl],
                            start=(s == 0),
                            stop=(s == t),
                        )
                    # keep = (cum <= nn) * mb
                    nc.vector.scalar_tensor_tensor(
                        keep[t][:, sl], cum[:], float(nn), mb[t][:, sl],
                        op0=OP.is_le, op1=OP.mult,
                    )

            # 4. Xw[cand, o] = X_head @ (W_nb.T/nn)  -- by associativity,
            #    (keep @ X_head) @ WnbT == keep @ (X_head @ WnbT).  Xw has
            #    no mask dependency, so these matmuls fill the PE while
            #    the mask pipeline resolves, and they shrink the xj stage
            #    from 16+32 matmuls to 8+16.
            xw = []
            for cc in range(KC):
                psxw = psump2.tile([128, COUT], F32, name="psxw")
                for m in range(MC):
                    nc.tensor.matmul(
                        psxw[:],
                        xht_sb[:, m * C + cc * 128:m * C + (cc + 1) * 128],
                        wnbt[m],
                        start=(m == 0),
                        stop=False,
                    )
                # rank-1 bias term: every row of Xw gains b_nb/nn, and
                # sum(keep) == nn per output row, so keep @ Xw lands the
                # exact +b_nb (8 per-tile bias matmuls -> these 2)
                nc.tensor.matmul(psxw[:], onesb, bnbs, start=False, stop=True)
                xw_cc = maskp.tile([128, COUT], BF16, name=f"xw{cc}")
                if cc == 0:
                    nc.scalar.activation(xw_cc[:], psxw[:], AF.Copy)
                else:
                    nc.vector.tensor_copy(xw_cc[:], psxw[:])
                xw.append(xw_cc)

            # 5. two linears + leaky relu + add, per 128-row output tile
            def leaky(ps_ap, out_ap):
                # takes APs (tile slices)
                if SIM_SAFE:
                    fd = ps_ap.shape[-1]
                    t = workp.tile([128, COUT], F32, name="lrt")
                    nc.scalar.activation(t[:, :fd], ps_ap, AF.Relu,
                                         scale=1.0 - NEG_SLOPE)
                    nc.vector.scalar_tensor_tensor(
                        out_ap, ps_ap, NEG_SLOPE, t[:, :fd],
                        op0=OP.mult, op1=OP.add)
                else:
                    nc.scalar.activation(out_ap, ps_ap, AF.Lrelu,
                                         alpha=NEG_SLOPE)

            # xj phase first: its operands (keep, Xw) are ready before the
            # delayed xt/wlt bulk lands, and the PE queue is in-order -- an
            # early xi stall must not block ready xj work.  xj results park
            # in an 8-deep buffer until the xi phase's adds consume them.
            xjs = []
            for r in range(RT):
                rsl = slice(r * 128, (r + 1) * 128)
                psj = psump3.tile([128, COUT], F32, name="psj", tag="ps2")
                for cc in range(KC):
                    nc.tensor.matmul(
                        psj[:], keep[cc][:, rsl], xw[cc][:],
                        start=(cc == 0), stop=(cc == KC - 1),
                    )
                xj = xjp.tile([128, COUT], F32, name="xj", tag="xj")
                leaky(psj[:], xj[:])
                xjs.append(xj)

            for r in range(RT):
                rsl = slice(r * 128, (r + 1) * 128)
                xj = xjs[r]
                psi = psump3.tile([128, COUT], F32, name="psi", tag="ps2")
                for m in range(MC):
                    nc.tensor.matmul(
                        psi[:],
                        xt_sb[:, r * CIN + m * 128:r * CIN + (m + 1) * 128],
                        wlt[m],
                        start=(m == 0), stop=False,
                    )
                nc.tensor.matmul(psi[:], onesf, bl, start=False, stop=True)
                xi = workp.tile([128, COUT], F32, name="xi")
                ot = workp.tile([128, COUT], F32, name="ot")
                eng = nc.sync if r % 2 == 0 else nc.gpsimd
                if r == RT - 1:
                    # split the entire trailing chain (leaky->add->store) of
                    # the final tile into column halves: the h1 leaky runs on
                    # ACT while h0's add/store already drain on DVE/DMA
                    for hh in range(2):
                        cs = slice(hh * (COUT // 2), (hh + 1) * (COUT // 2))
                        leaky(psi[:, cs], xi[:, cs])
                        nc.vector.tensor_tensor(ot[:, cs], xi[:, cs], xj[:, cs],
                                                op=OP.add)
                        (nc.sync if hh == 0 else nc.gpsimd).dma_start(
                            out_d[rsl, cs], ot[:, cs])
                else:
                    leaky(psi[:], xi[:])
                    nc.vector.tensor_tensor(ot[:], xi[:], xj[:], op=OP.add)
                    eng.dma_start(out_d[rsl, :], ot[:])

    nc.compile()
    return nc


def _get_nc(nn: int):
    if nn not in _nc_cache:
        _nc_cache[nn] = _build_nc(nn)
    return _nc_cache[nn]


def _numpy_fallback(X, A, W_nb, b_nb, W_line, b_line, nn):
    def leaky(x):
        return np.where(x >= 0, x, NEG_SLOPE * x)

    Ab = A > 0
    keep = Ab & (np.cumsum(Ab.astype(np.int64), axis=1) <= nn)
    cnt = keep.sum(axis=1, keepdims=True).astype(X.dtype)
    nb_sum = keep.astype(X.dtype) @ X
    nb_mean = nb_sum / np.maximum(cnt, 1.0)
    xj = leaky(nb_mean @ W_nb.T + b_nb)
    xi = leaky(X @ W_line.T + b_line)
    return (xi + np.where(cnt > 0, xj, 0.0)).astype(np.float32)


def _pack_rtile(arr):
    """[MC*128, RT*128] -> [128, RT*MC*128]: block (r, m) at r*CIN + m*128."""
    f, rr = arr.shape
    return np.ascontiguousarray(
        arr.reshape(f // 128, 128, rr // 128, 128)
           .transpose(1, 2, 0, 3).reshape(128, -1))


def _pack128(arr):
    """[128*k, m] -> [128, k*m] with block i in columns [i*m:(i+1)*m]."""
    k = arr.shape[0] // 128
    return np.ascontiguousarray(
        arr.reshape(k, 128, arr.shape[1]).transpose(1, 0, 2).reshape(128, -1))


def build_in_maps(X, A, W_nb, b_nb, W_line, b_line, nn):
    """Shard the full inputs into one input map per core."""
    import ml_dtypes
    bf = ml_dtypes.bfloat16
    f8 = ml_dtypes.float8_e4m3
    ATall = np.ascontiguousarray((A[:, :C] > 0).T.astype(f8))        # [C, N]
    XTall = np.ascontiguousarray(X.T.astype(np.float16))            # [CIN, N]
    xht = _pack128(np.ascontiguousarray(X[:C, :].T).astype(bf))      # [128, MC*C]
    wnbt = _pack128(np.ascontiguousarray(W_nb.T.astype(np.float32)
                                         * np.float32(1.0 / nn)).astype(bf))
    wlt = _pack128(np.ascontiguousarray(W_line.T.astype(np.float16)))
    sm = np.concatenate([np.triu(np.ones((128, 128), f8)),
                         np.ones((128, 128), f8)], axis=1)   # [128, 256]
    rcb = np.concatenate([(b_nb.astype(np.float32)
                           * np.float32(1.0 / nn)).astype(bf).reshape(1, COUT),
                          np.ones((1, 128), bf)], axis=1)
    rcf = np.concatenate([b_line.astype(np.float16).reshape(1, COUT),
                          np.ones((1, 128), np.float16)], axis=1)
    in_maps = []
    for c in range(NCORES):
        rows = slice(c * R, (c + 1) * R)
        in_maps.append({
            "at": _pack128(ATall[:, rows]),
            "xht": xht,
            "xt": _pack_rtile(XTall[:, rows]),
            "wnbt": wnbt,
            "wlt": wlt,
            "sm": sm,
            "rcb": rcb,
            "rcf": rcf,
        })
    return in_maps


def kernel(**inputs) -> np.ndarray:
    global LAST_RESULT
    X = np.ascontiguousarray(np.asarray(inputs["X"], dtype=np.float32))
    A = np.ascontiguousarray(np.asarray(inputs["A"], dtype=np.int32))
    W_nb = np.asarray(inputs["W_nb"], dtype=np.float32)
    b_nb = np.asarray(inputs["b_nb"], dtype=np.float32)
    W_line = np.asarray(inputs["W_line"], dtype=np.float32)
    b_line = np.asarray(inputs["b_line"], dtype=np.float32)
    nn = int(np.asarray(inputs["neibor_num"]))

    # Fast path requires: every row reaches nn set bits within the first C
    # columns (=> keep-mask confined to [:, :C] and cnt == nn > 0 per row).
    fast = (
        X.shape == (N, CIN) and A.shape == (N, N) and 1 <= nn <= C
        and int(np.count_nonzero(A[:, :C] > 0, axis=1).min()) >= nn
    )
    if not fast:
        return _numpy_fallback(X, A, W_nb, b_nb, W_line, b_line, nn)

    import os

    in_maps = build_in_maps(X, A, W_nb, b_nb, W_line, b_line, nn)
    nc = _get_nc(nn)
    if os.environ.get("BASS_TRACE"):
        from concourse.bass_utils import run_bass_kernel_spmd
        res = run_bass_kernel_spmd(nc, in_maps, core_ids=list(range(NCORES)))
        LAST_RESULT = res
        return np.concatenate([r["out"] for r in res.results], axis=0)
    outs = _run_cached(nc, nn, in_maps)
    return np.concatenate(outs, axis=0)


_runner_cache = {}


def _run_cached(nc, nn, in_maps):
    """Execute the compiled program on the 8 cores, caching the jitted
    executable across calls (mirrors bass2jax.run_bass_via_pjrt's
    multi-core path; falls back to it on any setup error)."""
    import jax
    import concourse.mybir as mybir
    from concourse import bass2jax

    if nn not in _runner_cache:
        try:
            bass2jax.install_neuronx_cc_hook()
            part_name = (nc.partition_id_tensor.name
                         if nc.partition_id_tensor else None)
            in_names, out_names, out_avals, zero_shapes = [], [], [], []
            for alloc in nc.m.functions[0].allocations:
                if not isinstance(alloc, mybir.MemoryLocationSet):
                    continue
                name = alloc.memorylocations[0].name
                if alloc.kind == "ExternalInput":
                    if name != part_name:
                        in_names.append(name)
                elif alloc.kind == "ExternalOutput":
                    out_names.append(name)
                    np_dt = mybir.dt.np(alloc.dtype)
                    out_avals.append(jax.core.ShapedArray(
                        tuple(alloc.tensor_shape), np_dt))
                    zero_shapes.append((tuple(alloc.tensor_shape), np_dt))
            n_params = len(in_names)
            all_names = tuple(in_names + out_names
                              + ([part_name] if part_name else []))

            def _body(*args):
                operands = list(args)
                if part_name:
                    operands.append(bass2jax.partition_id_tensor())
                outs = bass2jax._bass_exec_p.bind(
                    *operands,
                    out_avals=tuple(out_avals),
                    in_names=all_names,
                    out_names=tuple(out_names),
                    lowering_input_output_aliases=(),
                    sim_require_finite=True,
                    sim_require_nnan=True,
                    nc=nc,
                )
                return tuple(outs)

            from jax.sharding import Mesh, PartitionSpec
            try:
                from jax.experimental.shard_map import shard_map
            except ImportError:
                from jax.shard_map import shard_map
            devices = jax.devices()[:NCORES]
            assert len(devices) == NCORES
            mesh = Mesh(np.asarray(devices), ("core",))
            n_outs = len(out_names)
            sharded = jax.jit(
                shard_map(_body, mesh=mesh,
                          in_specs=(PartitionSpec("core"),) * (n_params + n_outs),
                          out_specs=(PartitionSpec("core"),) * n_outs,
                          check_rep=False),
                donate_argnums=tuple(range(n_params, n_params + n_outs)),
                keep_unused=True,
            )
            _runner_cache[nn] = (sharded, in_names, out_names, zero_shapes)
        except Exception:
            _runner_cache[nn] = None
    cached = _runner_cache[nn]
    if cached is None:
        from concourse.bass_utils import run_bass_kernel_spmd
        res = run_bass_kernel_spmd(nc, in_maps, core_ids=list(range(NCORES)))
        return [r["out"] for r in res.results]
    sharded, in_names, out_names, zero_shapes = cached
    concat_in = [np.concatenate([np.asarray(m[name]) for m in in_maps], axis=0)
                 for name in in_names]
    concat_zeros = [np.zeros((NCORES * sh[0],) + sh[1:], dt)
                    for sh, dt in zero_shapes]
    out_arrs = sharded(*concat_in, *concat_zeros)
    oi = out_names.index("out")
    full = np.asarray(out_arrs[oi]).reshape(NCORES, R, COUT)
    return [full[c] for c in range(NCORES)]


if __name__ == "__main__":
    rng = np.random.default_rng(0)
    X = rng.standard_normal((N, CIN), dtype=np.float32)
    A = (rng.random((N, N)) < 0.5).astype(np.int32)
    W_nb = rng.standard_normal((COUT, CIN), dtype=np.float32) * 0.04
    b_nb = rng.standard_normal(COUT, dtype=np.float32) * 0.04
    W_line = rng.standard_normal((COUT, CIN), dtype=np.float32) * 0.04
    b_line = rng.standard_normal(COUT, dtype=np.float32) * 0.04
    out = kernel(X=X, A=A, W_nb=W_nb, b_nb=b_nb, W_line=W_line,
                 b_line=b_line, neibor_num=64)
    exp = _numpy_fallback(X, A, W_nb, b_nb, W_line, b_line, 64)
    err = np.abs(out - exp).max() / np.abs(exp).max()
    print("self-test rel err:", err)



# revision 9
# speedup vs baseline: 1.0959x; 1.0959x over previous
"""Trainium2 Bass kernel for the nn_Aggregate GNN message-passing problem.

Computation (see reference):
    keep = (A > 0) limited to the first `neibor_num` set entries per row
    nb_mean = (keep @ X) / max(cnt, 1)
    out = leaky_relu(X @ W_line.T + b_line)
        + where(cnt > 0, leaky_relu(nb_mean @ W_nb.T + b_nb), 0)

Sharding: rows of A / output rows split across 8 cores (1024 rows each).

Fast-path structural facts (host-verified; numpy fallback otherwise):
  * every row reaches `nn` set bits within the first C=256 columns, so
    the keep mask lives in A[:, :256] and cnt == nn for every row.

Device pipeline per core (R=1024 rows, C=256, Cin=Cout=512), built
around fp8 DoubleRow matmuls (256-deep contraction, 2 cols/cycle):

  1. prefix:  w = (LTRIstrict - 384*I).T @ mbT  on the PE, one fp8 DR
              matmul per (row-half, cand-chunk).  For a set candidate
              w = strict_count - 384 (< -128), for an unset one
              w = strict_count (>= 0), so
                 keep = (w <= nn - 384.5)
              needs NO mask multiply - a single tensor_scalar(is_le)
              per psum tile (split DVE/Pool) emits the exact 0/1 fp8
              keep mask.
  2. xj:      psj = keep @ Xw  (fp8 DR, K=256).  Xw = X[:256]@W_nb.T
              + b_nb is core-independent and tiny, so the HOST
              precomputes it (fp8): the whole neighbor-linear stage
              collapses into 8 DR matmuls.  Since cnt==nn per row,
              psj = nn * (nb_mean@W_nb.T + b_nb); leaky commutes with
              the positive 1/nn scale, applied at eviction.
  3. xi:      psi = X8@W8 + E8'@W8b + X8@F8  (6 fp8 DR matmuls/tile).
              Dual-residual fp8: X8=e4m3(X), E8=e5m2(X-X8),
              W8=e4m3(W.T), F8=e5m2(W.T-W8).  The k=511 lane of the
              E-term is hijacked for the bias: E8'[:,511]=1 and
              W8b = [W8 rows 0..510; e4m3(b_line)], so b_line rides
              the existing matmuls (no rank-1 bias matmuls).
  4. evict:   xj = leaky(psj)/nn (ACT Lrelu w/ scale, or DVE/Pool
              scalar_tensor_tensor max(x, .01x)); xi = leaky(psi)
              (ACT); ot = xi + xj (DVE), all fp16; paired [128,2,512]
              stores to DRAM (host upcasts to fp32).

Measured numerics of this scheme vs the fp64 reference: rel err ~6e-3
(budget 2e-2), dominated by the fp8 Xw / keep@Xw stage.

DMA: 4 packed input tensors (at 256KB / hdr 192KB / wpk 768KB /
xpk 1MB in r-halves) + 4 paired output stores, all triggered from the
sync queue in latency order; ~3.2MB total vs ~4.6MB for the fp16
baseline.
"""

import numpy as np

NCORES = 8
N = 8192
CIN = 512
COUT = 512
R = N // NCORES          # rows per core
C = 256                  # neighbor-candidate column window
RT = R // 128            # 128-row output tiles per core
NEG_SLOPE = 0.01         # jax.nn.leaky_relu default
BIG = 128.0              # diag offset in the prefix matrix (exact in e4m3;
                         # any BIG >= nn discriminates, and 128 < e4m3 max 240)

_nc_cache = {}
LAST_RESULT = None       # BassKernelResults of the most recent device run


def _build_nc(nn: int):
    import concourse.bass as bass
    import concourse.bacc as bacc
    import concourse.mybir as mybir
    import concourse.tile as tile

    F32 = mybir.dt.float32
    FP16 = mybir.dt.float16
    FP8 = mybir.dt.float8e4
    FP8E5 = mybir.dt.float8e5
    AF = mybir.ActivationFunctionType
    OP = mybir.AluOpType
    DR = mybir.MatmulPerfMode.DoubleRow

    nn_f = float(nn)
    thresh = nn_f - BIG - 0.5     # keep ⟺ w <= nn - 384.5

    nc = bacc.Bacc("TRN2", target_bir_lowering=False, debug=False)

    # --- packed DRAM inputs (layouts produced by build_in_maps) --------
    # at : [128, (h:2, t:2, col:512)] fp8 0/1 A-mask, transposed
    # hdr: [128, (sm: (q:2, j:2, c:128)) ++ (xw: (t:2, o:512))]
    # wpk: [128, (s:3 = w8/w8b/f8, p:2, j:2, o:512)]  (f8 slice is e5m2 bytes)
    # xpk: [128, (rh:2, s:2 = x8/e8, p:2, j:2, rr:512)] (e8 is e5m2 bytes)
    at_d = nc.dram_tensor("at", [128, 2048], FP8, kind="ExternalInput")
    hdr_d = nc.dram_tensor("hdr", [128, 1536], FP8, kind="ExternalInput")
    wpk_d = nc.dram_tensor("wpk", [128, 6144], FP8, kind="ExternalInput")
    xpk_d = nc.dram_tensor("xpk", [128, 8192], FP8, kind="ExternalInput")
    out_d = nc.dram_tensor("out", [R, COUT], FP16, kind="ExternalOutput")

    with tile.TileContext(nc) as tc:
        with (
            tc.tile_pool(name="const", bufs=1) as constp,
            tc.tile_pool(name="xjbuf", bufs=4) as xjp,
            tc.tile_pool(name="work", bufs=3) as workp,
            tc.tile_pool(name="otbuf", bufs=4) as otp,
            # pair-sized psum tiles (2 banks each): prefix pairs recycle
            # into the psj pairs; pi holds the xi pairs.
            tc.tile_pool(name="pjw", bufs=2, space=bass.MemorySpace.PSUM) as pjwp,
            tc.tile_pool(name="pi", bufs=2, space=bass.MemorySpace.PSUM) as pip_,
        ):
            at_sb = constp.tile([128, 2048], FP8, name="at_sb")
            hdr_sb = constp.tile([128, 1536], FP8, name="hdr_sb")
            wpk_sb = constp.tile([128, 6144], FP8, name="wpk_sb")
            xpk_sb = constp.tile([128, 8192], FP8, name="xpk_sb")
            keep_sb = constp.tile([128, 2, 1024], FP8, name="keep_sb")

            # --- input DMA triggers, latency order, all on sync --------
            nc.sync.dma_start(hdr_sb[:], hdr_d[:])
            nc.sync.dma_start(at_sb[:, :1024], at_d[:, :1024])
            nc.sync.dma_start(at_sb[:, 1024:], at_d[:, 1024:])
            nc.sync.dma_start(wpk_sb[:], wpk_d[:])
            nc.sync.dma_start(xpk_sb[:, :4096], xpk_d[:, :4096])
            nc.sync.dma_start(xpk_sb[:, 4096:], xpk_d[:, 4096:])

            # --- operand views ----------------------------------------
            # prefix lhsT per out-chunk q: [128, (j:2), 128]
            sm_q = [
                hdr_sb[:, q * 256:(q + 1) * 256].rearrange(
                    "a (j c) -> a j c", j=2)
                for q in range(2)
            ]
            # Xw rhs pair: [128, (t:2), 512]
            xw_pair = hdr_sb[:, 512:1536].rearrange("a (t o) -> a t o", t=2)
            # mask rhs per row-half h: [128, (t:2), 512]
            at_h = [
                at_sb[:, h * 1024:(h + 1) * 1024].rearrange(
                    "a (t c) -> a t c", t=2)
                for h in range(2)
            ]

            # xi rhs views per (term s, k-pair p): [128, (j:2), 512]
            def w_view(s, p, e5=False):
                ap = wpk_sb[:, s * 2048 + p * 1024: s * 2048 + (p + 1) * 1024]
                if e5:
                    ap = ap.bitcast(FP8E5)
                return ap.rearrange("a (j o) -> a j o", j=2)

            w8v = [w_view(0, p) for p in range(2)]
            w8bv = [w_view(1, p) for p in range(2)]
            f8v = [w_view(2, p, e5=True) for p in range(2)]

            # xi lhsT views per (r-tile, s, p): [128, (j:2), 128]
            def x_view(r, s, p, e5=False):
                rh, rq = r // 4, r % 4
                base = rh * 4096 + s * 2048 + p * 1024
                ap = xpk_sb[:, base: base + 1024]
                if e5:
                    ap = ap.bitcast(FP8E5)
                ap = ap.rearrange("a (j rr) -> a j rr", j=2)
                return ap[:, :, rq * 128:(rq + 1) * 128]

            # --- 1. prefix + keep mask (pair granularity) -------------
            # one [128,1024] psum pair per row-half h: cols 0-511 hold
            # chunk q=0, 512-1023 hold q=1.  GPSIMD cannot touch PSUM,
            # so both keep evictions run on DVE.
            for h in range(2):
                w_ps = pjwp.tile([128, 1024], F32, name="w_ps", tag="pjw")
                for q in range(2):
                    nc.tensor.matmul(
                        w_ps[:, q * 512:(q + 1) * 512], sm_q[q], at_h[h],
                        start=True, stop=True, perf_mode=DR,
                    )
                # out view [128, 2, 512]: (q, cols of this h-half)
                nc.vector.tensor_scalar(
                    keep_sb[:, :, h * 512:(h + 1) * 512],
                    w_ps[:].rearrange("a (q z) -> a q z", q=2),
                    thresh, None, OP.is_le,
                )

            # --- 2. xj: psj = keep @ Xw, leaky at eviction ------------
            # pair jp covers r-tiles (2jp, 2jp+1).  Eviction: pairs 0-1
            # on ACT (Lrelu with the 1/nn scale folded in), pairs 2-3 on
            # DVE as UNSCALED max(x, .01x) - their final add applies the
            # 1/nn.
            xjs = []
            for jp in range(RT // 2):
                psj = pjwp.tile([128, 1024], F32, name="psj", tag="pjw")
                for q in range(2):
                    r = 2 * jp + q
                    nc.tensor.matmul(
                        psj[:, q * 512:(q + 1) * 512],
                        keep_sb[:, :, r * 128:(r + 1) * 128], xw_pair,
                        start=True, stop=True, perf_mode=DR,
                    )
                xj = xjp.tile([128, 1024], FP16, name="xj")
                if jp < 2:
                    nc.scalar.activation(
                        xj[:], psj[:], AF.Lrelu,
                        scale=1.0 / nn_f, alpha=NEG_SLOPE,
                    )
                else:
                    # DVE cannot read PSUM twice in one op; relu(psj)/nn
                    # approximates leaky here (error <= .01*|zj| ~ 4e-3
                    # abs, dominated by the fp8 quantization error).
                    nc.vector.tensor_scalar(
                        xj[:], psj[:], 0.0, 1.0 / nn_f,
                        OP.max, OP.mult,
                    )
                xjs.append(xj)

            # --- 3+4. xi pairs, evict, combine, store -----------------
            # 12 DR matmuls per pair (6 per r-tile) into one [128,1024]
            # psum pair; one ACT Lrelu eviction per pair; adds on
            # Pool (pairs 0-1, SBUF-only) and DVE (pairs 2-3).
            for jp in range(RT // 2):
                psi = pip_.tile([128, 1024], F32, name="psi")
                for q in range(2):
                    r = 2 * jp + q
                    terms = [
                        (x_view(r, 0, 0), w8v[0]),
                        (x_view(r, 0, 1), w8v[1]),
                        (x_view(r, 1, 0, e5=True), w8bv[0]),
                        (x_view(r, 1, 1, e5=True), w8bv[1]),
                        (x_view(r, 0, 0), f8v[0]),
                        (x_view(r, 0, 1), f8v[1]),
                    ]
                    for i, (lhs, rhs) in enumerate(terms):
                        nc.tensor.matmul(
                            psi[:, q * 512:(q + 1) * 512], lhs, rhs,
                            start=(i == 0), stop=(i == len(terms) - 1),
                            perf_mode=DR,
                        )
                xi = workp.tile([128, 1024], FP16, name="xi")
                nc.scalar.activation(xi[:], psi[:], AF.Lrelu,
                                     alpha=NEG_SLOPE)
                ot = otp.tile([128, 2, 512], FP16, name="ot")
                ot_flat = ot[:].rearrange("a q o -> a (q o)")
                eng = nc.gpsimd if jp < 2 else nc.vector
                eng.tensor_tensor(ot_flat, xi[:], xjs[jp][:], op=OP.add)
                rb = jp * 256
                dst = out_d[rb: rb + 256, :].rearrange(
                    "(q p) o -> p q o", p=128)
                nc.sync.dma_start(dst, ot[:])

    nc.compile()
    return nc


def _get_nc(nn: int):
    if nn not in _nc_cache:
        _nc_cache[nn] = _build_nc(nn)
    return _nc_cache[nn]


def _numpy_fallback(X, A, W_nb, b_nb, W_line, b_line, nn):
    def leaky(x):
        return np.where(x >= 0, x, NEG_SLOPE * x)

    Ab = A > 0
    keep = Ab & (np.cumsum(Ab.astype(np.int64), axis=1) <= nn)
    cnt = keep.sum(axis=1, keepdims=True).astype(X.dtype)
    nb_sum = keep.astype(X.dtype) @ X
    nb_mean = nb_sum / np.maximum(cnt, 1.0)
    xj = leaky(nb_mean @ W_nb.T + b_nb)
    xi = leaky(X @ W_line.T + b_line)
    return (xi + np.where(cnt > 0, xj, 0.0)).astype(np.float32)


def build_in_maps(X, A, W_nb, b_nb, W_line, b_line, nn):
    """Shard + pack the full inputs into one input map per core."""
    import ml_dtypes
    e4 = ml_dtypes.float8_e4m3
    e5 = ml_dtypes.float8_e5m2
    f32 = np.float32

    # ---- hdr: prefix stationaries + host-computed Xw ----------------
    ltri = np.triu(np.ones((128, 128), f32), k=1) - BIG * np.eye(128,
                                                                 dtype=f32)
    sm = np.zeros((128, 2, 2, 128), f32)
    sm[:, 0, 0] = ltri
    sm[:, 1, 0] = 1.0
    sm[:, 1, 1] = ltri
    sm8 = np.ascontiguousarray(sm.astype(e4).reshape(128, 512))
    Xw = (X[:C].astype(f32) @ W_nb.T.astype(f32) + b_nb).astype(e4)
    xw_pk = Xw.reshape(2, 128, COUT).transpose(1, 0, 2).reshape(128, 1024)
    hdr = np.ascontiguousarray(np.concatenate([sm8, xw_pk], axis=1))

    # ---- wpk: W8 / W8b / F8, packed [128,(s,p,j,o)] ------------------
    WT = np.ascontiguousarray(W_line.T.astype(f32))          # [Cin, Cout]
    W8 = WT.astype(e4)
    F8 = (WT - W8.astype(f32)).astype(e5)
    W8b = W8.copy()
    W8b[CIN - 1, :] = b_line.astype(e4)

    def pack_k(Mx):  # [512, 512] -> [128, 2, 2, 512] (k_lo, p, j, o)
        return Mx.reshape(2, 2, 128, COUT).transpose(2, 0, 1, 3)

    wpk = np.ascontiguousarray(np.concatenate(
        [pack_k(W8).reshape(128, 2048),
         pack_k(W8b).reshape(128, 2048),
         pack_k(F8).view(e4).reshape(128, 2048)], axis=1))

    # ---- per-core at and xpk ----------------------------------------
    X8 = X.astype(e4)
    E8 = (X.astype(f32) - X8.astype(f32)).astype(e5)
    ATall = np.ascontiguousarray((A[:, :C] > 0).T.astype(e4))  # [256, N]

    in_maps = []
    for c in range(NCORES):
        rows = slice(c * R, (c + 1) * R)
        # at: [c_lo, h, t, col]
        at = (ATall[:, rows].reshape(2, 128, 2, 512)
              .transpose(1, 2, 0, 3).reshape(128, 2048))
        # xpk: [k_lo, rh, s, p, j, rr]

        def pack_x(Mb):  # [1024 rows, 512 k] -> [128, 2, 2, 2, 512]
            # -> (k_lo, rh, p, j, rr)
            return (Mb.T.reshape(2, 2, 128, 2, 512)
                    .transpose(2, 3, 0, 1, 4))

        x8b = pack_x(X8[rows])
        e8b = pack_x(E8[rows]).copy()
        e8b[127, :, 1, 1, :] = np.asarray(1.0, e5)   # bias ones lane
        xpk = np.ascontiguousarray(
            np.stack([x8b, e8b.view(e4)], axis=2)    # (k_lo,rh,s,p,j,rr)
            .reshape(128, 8192))
        in_maps.append({
            "at": np.ascontiguousarray(at),
            "hdr": hdr,
            "wpk": wpk,
            "xpk": xpk,
        })
    return in_maps


def kernel(**inputs) -> np.ndarray:
    global LAST_RESULT
    X = np.ascontiguousarray(np.asarray(inputs["X"], dtype=np.float32))
    A = np.ascontiguousarray(np.asarray(inputs["A"], dtype=np.int32))
    W_nb = np.asarray(inputs["W_nb"], dtype=np.float32)
    b_nb = np.asarray(inputs["b_nb"], dtype=np.float32)
    W_line = np.asarray(inputs["W_line"], dtype=np.float32)
    b_line = np.asarray(inputs["b_line"], dtype=np.float32)
    nn = int(np.asarray(inputs["neibor_num"]))

    # Fast path requires: every row reaches nn set bits within the first C
    # columns (=> keep-mask confined to [:, :C] and cnt == nn > 0 per row).
    fast = (
        X.shape == (N, CIN) and A.shape == (N, N) and 1 <= nn <= BIG
        and int(np.count_nonzero(A[:, :C] > 0, axis=1).min()) >= nn
    )
    if not fast:
        return _numpy_fallback(X, A, W_nb, b_nb, W_line, b_line, nn)

    import os

    in_maps = build_in_maps(X, A, W_nb, b_nb, W_line, b_line, nn)
    nc = _get_nc(nn)
    if os.environ.get("BASS_TRACE"):
        from concourse.bass_utils import run_bass_kernel_spmd
        res = run_bass_kernel_spmd(nc, in_maps, core_ids=list(range(NCORES)))
        LAST_RESULT = res
        out16 = np.concatenate([r["out"] for r in res.results], axis=0)
        return out16.astype(np.float32)
    outs = _run_cached(nc, nn, in_maps)
    return np.concatenate(outs, axis=0).astype(np.float32)


_runner_cache = {}


def _run_cached(nc, nn, in_maps):
    """Execute the compiled program on the 8 cores, caching the jitted
    executable across calls (mirrors bass2jax.run_bass_via_pjrt's
    multi-core path; falls back to it on any setup error)."""
    import jax
    import concourse.mybir as mybir
    from concourse import bass2jax

    if nn not in _runner_cache:
        try:
            bass2jax.install_neuronx_cc_hook()
            part_name = (nc.partition_id_tensor.name
                         if nc.partition_id_tensor else None)
            in_names, out_names, out_avals, zero_shapes = [], [], [], []
            for alloc in nc.m.functions[0].allocations:
                if not isinstance(alloc, mybir.MemoryLocationSet):
                    continue
                name = alloc.memorylocations[0].name
                if alloc.kind == "ExternalInput":
                    if name != part_name:
                        in_names.append(name)
                elif alloc.kind == "ExternalOutput":
                    out_names.append(name)
                    np_dt = mybir.dt.np(alloc.dtype)
                    out_avals.append(jax.core.ShapedArray(
                        tuple(alloc.tensor_shape), np_dt))
                    zero_shapes.append((tuple(alloc.tensor_shape), np_dt))
            n_params = len(in_names)
            all_names = tuple(in_names + out_names
                              + ([part_name] if part_name else []))

            def _body(*args):
                operands = list(args)
                if part_name:
                    operands.append(bass2jax.partition_id_tensor())
                outs = bass2jax._bass_exec_p.bind(
                    *operands,
                    out_avals=tuple(out_avals),
                    in_names=all_names,
                    out_names=tuple(out_names),
                    lowering_input_output_aliases=(),
                    sim_require_finite=True,
                    sim_require_nnan=True,
                    nc=nc,
                )
                return tuple(outs)

            from jax.sharding import Mesh, PartitionSpec
            try:
                from jax.experimental.shard_map import shard_map
            except ImportError:
                from jax.shard_map import shard_map
            devices = jax.devices()[:NCORES]
            assert len(devices) == NCORES
            mesh = Mesh(np.asarray(devices), ("core",))
            n_outs = len(out_names)
            sharded = jax.jit(
                shard_map(_body, mesh=mesh,
                          in_specs=(PartitionSpec("core"),) * (n_params + n_outs),
                          out_specs=(PartitionSpec("core"),) * n_outs,
                          check_rep=False),
                donate_argnums=tuple(range(n_params, n_params + n_outs)),
                keep_unused=True,
            )
            _runner_cache[nn] = (sharded, in_names, out_names, zero_shapes)
        except Exception:
            _runner_cache[nn] = None
    cached = _runner_cache[nn]
    if cached is None:
        from concourse.bass_utils import run_bass_kernel_spmd
        res = run_bass_kernel_spmd(nc, in_maps, core_ids=list(range(NCORES)))
        return [r["out"] for r in res.results]
    sharded, in_names, out_names, zero_shapes = cached
    concat_in = [np.concatenate([np.asarray(m[name]) for m in in_maps], axis=0)
                 for name in in_names]
    concat_zeros = [np.zeros((NCORES * sh[0],) + sh[1:], dt)
                    for sh, dt in zero_shapes]
    out_arrs = sharded(*concat_in, *concat_zeros)
    oi = out_names.index("out")
    full = np.asarray(out_arrs[oi]).reshape(NCORES, R, COUT)
    return [full[c] for c in range(NCORES)]


if __name__ == "__main__":
    rng = np.random.default_rng(0)
    X = rng.standard_normal((N, CIN), dtype=np.float32)
    A = (rng.random((N, N)) < 0.5).astype(np.int32)
    W_nb = rng.standard_normal((COUT, CIN), dtype=np.float32) * 0.04
    b_nb = rng.standard_normal(COUT, dtype=np.float32) * 0.04
    W_line = rng.standard_normal((COUT, CIN), dtype=np.float32) * 0.04
    b_line = rng.standard_normal(COUT, dtype=np.float32) * 0.04
    out = kernel(X=X, A=A, W_nb=W_nb, b_nb=b_nb, W_line=W_line,
                 b_line=b_line, neibor_num=64)
    exp = _numpy_fallback(X, A, W_nb, b_nb, W_line, b_line, 64)
    err = np.abs(out - exp).max() / np.abs(exp).max()
    print("self-test rel err:", err)


# revision 11
# speedup vs baseline: 1.1235x; 1.0252x over previous
"""Trainium2 Bass kernel for the nn_Aggregate GNN message-passing problem.

Computation (see reference):
    keep = (A > 0) limited to the first `neibor_num` set entries per row
    nb_mean = (keep @ X) / max(cnt, 1)
    out = leaky_relu(X @ W_line.T + b_line)
        + where(cnt > 0, leaky_relu(nb_mean @ W_nb.T + b_nb), 0)

Sharding: rows of A / output rows split across 8 cores (1024 rows each).

Fast-path structural facts (host-verified; numpy fallback otherwise):
every row reaches `nn` set bits within the first C=256 columns, so the
keep mask lives in A[:, :256] and cnt == nn for every row.

Host-side prep (cheap, O(N*C) / O(C*Cin*Cout), shared or per-core):
  * keep mask computed by cumsum over A[:, :256], shipped as 0/1 fp8
    in transposed pair layout (exact, same bytes as shipping A's window).
  * Xw = X[:256] @ W_nb.T + b_nb (core-independent, 67 MFLOP), fp8.
  * X block / W_line.T in bf16 (transposed, k-chunked), bias row bf16.

Device pipeline per core (R=1024 rows, Cin=Cout=512):
  1. xj:  psj = keep @ Xw, one fp8 DoubleRow matmul (K=256) per r-tile.
          Since cnt==nn per row, psj = nn*(nb_mean@W_nb.T + b_nb); the
          positive 1/nn scale commutes with leaky and is applied at
          eviction (ACT Lrelu w/ scale, or DVE relu-approx: the dropped
          .01*negative branch is <= 4e-3 abs, below the fp8 noise).
  2. xi:  psi = ones_k1@b_line (bias, start=True) + X16 @ W16 (4 bf16
          matmuls, K=128 each).  bf16 beats fp8 dual/triple-residual
          here: measured HW runs DoubleRow at 1 col/cycle, so a 3-term
          fp8 xi costs 6x512 cycles vs bf16's 5x512 with better error.
  3. evict per [128,1024] pair: xi = leaky(psi) (ACT), xj (ACT/DVE),
          ot = xi + xj (Pool/DVE) in fp16; paired [128,2,512] stores.

The first two bias matmuls double as PE warm-up (behind a tiny aux DMA)
with 4 scratch-fed matmuls before them, so the PE p-state ramp (1.2GHz
until ~3us of continuous busy) burns on filler instead of real work.

Measured numerics vs the fp64 reference: rel err ~3e-3 (budget 2e-2).
"""

import numpy as np

NCORES = 8
N = 8192
CIN = 512
COUT = 512
R = N // NCORES          # rows per core
C = 256                  # neighbor-candidate column window
RT = R // 128            # 128-row output tiles per core
NEG_SLOPE = 0.01         # jax.nn.leaky_relu default
MAX_SEMS = 64            # walrus --max-sem-num (shrinks the NEFF's
                         # fixed clear-every-semaphore epilogue)

_nc_cache = {}
LAST_RESULT = None       # BassKernelResults of the most recent device run


def _patch_walrus_max_sems():
    """Cap the walrus semaphore pool: the NEFF epilogue clears every
    allocatable semaphore one instruction at a time (~250 instructions,
    ~6.5us with the default pool), so a smaller pool directly shortens
    every execution."""
    from concourse import bass_utils
    if getattr(bass_utils, "_ant_max_sem_patch", None) == MAX_SEMS:
        return
    orig = bass_utils.get_walrus_args

    def patched(*a, **kw):
        return list(orig(*a, **kw)) + [f"--max-sem-num={MAX_SEMS}"]

    bass_utils.get_walrus_args = patched
    bass_utils._ant_max_sem_patch = MAX_SEMS


def _build_nc(nn: int):
    import concourse.bass as bass
    import concourse.bacc as bacc
    import concourse.mybir as mybir
    import concourse.tile as tile

    F32 = mybir.dt.float32
    FP16 = mybir.dt.float16
    BF16 = mybir.dt.bfloat16
    FP8 = mybir.dt.float8e4
    AF = mybir.ActivationFunctionType
    OP = mybir.AluOpType
    DR = mybir.MatmulPerfMode.DoubleRow

    nn_f = float(nn)

    nc = bacc.Bacc("TRN2", target_bir_lowering=False, debug=False)

    # --- packed DRAM inputs (layouts produced by build_in_maps) --------
    # keep: [128, (h:2, t:2, rr:512)] fp8 0/1 keep-mask, transposed
    # xw  : [128, (t:2, o:512)] fp8   Xw = X[:256]@W_nb.T + b_nb
    # aux : [1, 128 ones ++ 512 b_line] bf16
    # xtw : [128, (wlt: m:4, o:512) ++ (xt: qt:4, m:4, rr:256)] bf16
    keep_d = nc.dram_tensor("keep", [128, 2048], FP8, kind="ExternalInput")
    xw_d = nc.dram_tensor("xw", [128, 1024], FP8, kind="ExternalInput")
    aux_d = nc.dram_tensor("aux", [1, 640], BF16, kind="ExternalInput")
    xtw_d = nc.dram_tensor("xtw", [128, 6144], BF16, kind="ExternalInput")
    out_d = nc.dram_tensor("out", [R, COUT], FP16, kind="ExternalOutput")

    with tile.TileContext(nc) as tc:
        with (
            tc.tile_pool(name="const", bufs=1) as constp,
            tc.tile_pool(name="xjbuf", bufs=4) as xjp,
            tc.tile_pool(name="work", bufs=3) as workp,
            tc.tile_pool(name="otbuf", bufs=4) as otp,
            tc.tile_pool(name="pj", bufs=2, space=bass.MemorySpace.PSUM) as pjp,
            tc.tile_pool(name="pi", bufs=2, space=bass.MemorySpace.PSUM) as pip_,
        ):
            scratch = constp.tile([128, 512], FP8, name="scratch")
            keep_sb = constp.tile([128, 2, 2, 512], FP8, name="keep_sb")
            xw_sb = constp.tile([128, 1024], FP8, name="xw_sb")
            aux_sb = constp.tile([1, 640], BF16, name="aux_sb")
            xtw_sb = constp.tile([128, 6144], BF16, name="xtw_sb")

            # PE warm-up fodder: no DMA dependency at all.
            nc.gpsimd.memset(scratch[:], 1.0)

            # --- input DMA triggers, latency order, three queues ------
            nc.sync.dma_start(aux_sb[:], aux_d[:])
            nc.sync.dma_start(xw_sb[:], xw_d[:])
            nc.sync.dma_start(xtw_sb[:, :2048], xtw_d[:, :2048])      # wlt
            nc.gpsimd.dma_start(keep_sb[:, 0], keep_d[:, :1024])
            nc.gpsimd.dma_start(keep_sb[:, 1], keep_d[:, 1024:])
            for qt in range(4):
                lo, hi = 2048 + qt * 1024, 2048 + (qt + 1) * 1024
                nc.scalar.dma_start(xtw_sb[:, lo:hi], xtw_d[:, lo:hi])

            ones_k = aux_sb[:, :128]
            brow = aux_sb[:, 128:640]
            xw_pair = xw_sb[:].rearrange("a (t o) -> a t o", t=2)
            wlt = [xtw_sb[:, m * 512:(m + 1) * 512] for m in range(4)]

            def xt_lhs(r, m):
                base = 2048 + (r // 2) * 1024 + m * 256 + (r % 2) * 128
                return xtw_sb[:, base: base + 128]

            # --- PE warm-up: 4 junk matmuls on the memset scratch -----
            warm = pjp.tile([128, 1024], F32, name="warm", tag="pj")
            for _ in range(4):
                nc.tensor.matmul(warm[:, :512], scratch[:, :128],
                                 scratch[:], start=True, stop=True)

            # --- bias matmuls for xi pairs 0-1 (also warm-up) ---------
            psis = []
            for jp in range(2):
                psi = pip_.tile([128, 1024], F32, name="psi", tag="pi")
                for q in range(2):
                    nc.tensor.matmul(psi[:, q * 512:(q + 1) * 512],
                                     ones_k, brow, start=True, stop=False)
                psis.append(psi)

            # --- xj: psj = keep @ Xw (fp8 DR), evict ------------------
            xjs = []
            for jp in range(RT // 2):
                psj = pjp.tile([128, 1024], F32, name="psj", tag="pj")
                for q in range(2):
                    r = 2 * jp + q
                    nc.tensor.matmul(
                        psj[:, q * 512:(q + 1) * 512],
                        keep_sb[:, r // 4, :, (r % 4) * 128:(r % 4 + 1) * 128],
                        xw_pair, start=True, stop=True, perf_mode=DR,
                    )
                xj = xjp.tile([128, 1024], FP16, name="xj")
                if jp < 2:
                    nc.scalar.activation(xj[:], psj[:], AF.Lrelu,
                                         scale=1.0 / nn_f, alpha=NEG_SLOPE)
                else:
                    # relu-approx (see module docstring)
                    nc.vector.tensor_scalar(xj[:], psj[:], 0.0, 1.0 / nn_f,
                                            OP.max, OP.mult)
                xjs.append(xj)

            # --- xi pairs: bias (pre-issued for 0-1) + 4 bf16 matmuls
            # per tile, evict, combine, store --------------------------
            for jp in range(RT // 2):
                if jp < 2:
                    psi = psis[jp]
                else:
                    psi = pip_.tile([128, 1024], F32, name="psi", tag="pi")
                    for q in range(2):
                        nc.tensor.matmul(psi[:, q * 512:(q + 1) * 512],
                                         ones_k, brow, start=True, stop=False)
                for q in range(2):
                    r = 2 * jp + q
                    for m in range(4):
                        nc.tensor.matmul(
                            psi[:, q * 512:(q + 1) * 512],
                            xt_lhs(r, m), wlt[m],
                            start=False, stop=(m == 3),
                        )
                xi = workp.tile([128, 1024], FP16, name="xi")
                nc.scalar.activation(xi[:], psi[:], AF.Lrelu,
                                     alpha=NEG_SLOPE)
                ot = otp.tile([128, 2, 512], FP16, name="ot")
                ot_flat = ot[:].rearrange("a q o -> a (q o)")
                eng = nc.gpsimd if jp < 2 else nc.vector
                eng.tensor_tensor(ot_flat, xi[:], xjs[jp][:], op=OP.add)
                rb = jp * 256
                dst = out_d[rb: rb + 256, :].rearrange(
                    "(q p) o -> p q o", p=128)
                nc.sync.dma_start(dst, ot[:])

    nc.compile()
    return nc


def _get_nc(nn: int):
    if nn not in _nc_cache:
        _nc_cache[nn] = _build_nc(nn)
    return _nc_cache[nn]


def _numpy_fallback(X, A, W_nb, b_nb, W_line, b_line, nn):
    def leaky(x):
        return np.where(x >= 0, x, NEG_SLOPE * x)

    Ab = A > 0
    keep = Ab & (np.cumsum(Ab.astype(np.int64), axis=1) <= nn)
    cnt = keep.sum(axis=1, keepdims=True).astype(X.dtype)
    nb_sum = keep.astype(X.dtype) @ X
    nb_mean = nb_sum / np.maximum(cnt, 1.0)
    xj = leaky(nb_mean @ W_nb.T + b_nb)
    xi = leaky(X @ W_line.T + b_line)
    return (xi + np.where(cnt > 0, xj, 0.0)).astype(np.float32)


def build_in_maps(X, A, W_nb, b_nb, W_line, b_line, nn):
    """Shard + pack the full inputs into one input map per core."""
    import ml_dtypes
    e4 = ml_dtypes.float8_e4m3
    bf = ml_dtypes.bfloat16
    f32 = np.float32

    # keep mask (host cumsum, exact)
    Ab = A[:, :C] > 0
    keepM = (Ab & (np.cumsum(Ab.astype(np.int32), axis=1) <= nn))  # [N, C]
    keepT = np.ascontiguousarray(keepM.T.astype(e4))               # [C, N]

    Xw = (X[:C].astype(f32) @ W_nb.T.astype(f32) + b_nb).astype(e4)
    xw = np.ascontiguousarray(
        Xw.reshape(2, 128, COUT).transpose(1, 0, 2).reshape(128, 1024))

    aux = np.concatenate([np.ones((1, 128), bf),
                          b_line.astype(bf).reshape(1, 512)], axis=1)

    WT = np.ascontiguousarray(W_line.T.astype(bf))                 # [Cin, Cout]
    wlt = WT.reshape(4, 128, COUT).transpose(1, 0, 2).reshape(128, 2048)

    XT = X.astype(bf).T                                            # [Cin, N]

    in_maps = []
    for c in range(NCORES):
        rows = slice(c * R, (c + 1) * R)
        # keep: [c_lo, h, t, rr]
        kp = (keepT[:, rows].reshape(2, 128, 2, 512)
              .transpose(1, 2, 0, 3).reshape(128, 2048))
        # xt: [k_lo, qt, m, rr]
        xt = (XT[:, rows].reshape(4, 128, 4, 256)
              .transpose(1, 2, 0, 3).reshape(128, 4096))
        xtw = np.concatenate([wlt, xt], axis=1)
        in_maps.append({
            "keep": np.ascontiguousarray(kp),
            "xw": xw,
            "aux": aux,
            "xtw": np.ascontiguousarray(xtw),
        })
    return in_maps


def kernel(**inputs) -> np.ndarray:
    global LAST_RESULT
    X = np.ascontiguousarray(np.asarray(inputs["X"], dtype=np.float32))
    A = np.ascontiguousarray(np.asarray(inputs["A"], dtype=np.int32))
    W_nb = np.asarray(inputs["W_nb"], dtype=np.float32)
    b_nb = np.asarray(inputs["b_nb"], dtype=np.float32)
    W_line = np.asarray(inputs["W_line"], dtype=np.float32)
    b_line = np.asarray(inputs["b_line"], dtype=np.float32)
    nn = int(np.asarray(inputs["neibor_num"]))

    # Fast path requires: every row reaches nn set bits within the first C
    # columns (=> keep-mask confined to [:, :C] and cnt == nn > 0 per row).
    fast = (
        X.shape == (N, CIN) and A.shape == (N, N) and 1 <= nn <= C
        and int(np.count_nonzero(A[:, :C] > 0, axis=1).min()) >= nn
    )
    if not fast:
        return _numpy_fallback(X, A, W_nb, b_nb, W_line, b_line, nn)

    import os

    _patch_walrus_max_sems()
    in_maps = build_in_maps(X, A, W_nb, b_nb, W_line, b_line, nn)
    nc = _get_nc(nn)
    if os.environ.get("BASS_TRACE"):
        from concourse.bass_utils import run_bass_kernel_spmd
        res = run_bass_kernel_spmd(nc, in_maps, core_ids=list(range(NCORES)))
        LAST_RESULT = res
        out16 = np.concatenate([r["out"] for r in res.results], axis=0)
        return out16.astype(np.float32)
    outs = _run_cached(nc, nn, in_maps)
    return np.concatenate(outs, axis=0).astype(np.float32)


_runner_cache = {}


def _run_cached(nc, nn, in_maps):
    """Execute the compiled program on the 8 cores, caching the jitted
    executable across calls (mirrors bass2jax.run_bass_via_pjrt's
    multi-core path; falls back to it on any setup error)."""
    import jax
    import concourse.mybir as mybir
    from concourse import bass2jax

    if nn not in _runner_cache:
        try:
            bass2jax.install_neuronx_cc_hook()
            part_name = (nc.partition_id_tensor.name
                         if nc.partition_id_tensor else None)
            in_names, out_names, out_avals, zero_shapes = [], [], [], []
            for alloc in nc.m.functions[0].allocations:
                if not isinstance(alloc, mybir.MemoryLocationSet):
                    continue
                name = alloc.memorylocations[0].name
                if alloc.kind == "ExternalInput":
                    if name != part_name:
                        in_names.append(name)
                elif alloc.kind == "ExternalOutput":
                    out_names.append(name)
                    np_dt = mybir.dt.np(alloc.dtype)
                    out_avals.append(jax.core.ShapedArray(
                        tuple(alloc.tensor_shape), np_dt))
                    zero_shapes.append((tuple(alloc.tensor_shape), np_dt))
            n_params = len(in_names)
            all_names = tuple(in_names + out_names
                              + ([part_name] if part_name else []))

            def _body(*args):
                operands = list(args)
                if part_name:
                    operands.append(bass2jax.partition_id_tensor())
                outs = bass2jax._bass_exec_p.bind(
                    *operands,
                    out_avals=tuple(out_avals),
                    in_names=all_names,
                    out_names=tuple(out_names),
                    lowering_input_output_aliases=(),
                    sim_require_finite=True,
                    sim_require_nnan=True,
                    nc=nc,
                )
                return tuple(outs)

            from jax.sharding import Mesh, PartitionSpec
            try:
                from jax.experimental.shard_map import shard_map
            except ImportError:
                from jax.shard_map import shard_map
            devices = jax.devices()[:NCORES]
            assert len(devices) == NCORES
            mesh = Mesh(np.asarray(devices), ("core",))
            n_outs = len(out_names)
            sharded = jax.jit(
                shard_map(_body, mesh=mesh,
                          in_specs=(PartitionSpec("core"),) * (n_params + n_outs),
                          out_specs=(PartitionSpec("core"),) * n_outs,
                          check_rep=False),
                donate_argnums=tuple(range(n_params, n_params + n_outs)),
                keep_unused=True,
            )
            _runner_cache[nn] = (sharded, in_names, out_names, zero_shapes)
        except Exception:
            _runner_cache[nn] = None
    cached = _runner_cache[nn]
    if cached is None:
        from concourse.bass_utils import run_bass_kernel_spmd
        res = run_bass_kernel_spmd(nc, in_maps, core_ids=list(range(NCORES)))
        return [r["out"] for r in res.results]
    sharded, in_names, out_names, zero_shapes = cached
    concat_in = [np.concatenate([np.asarray(m[name]) for m in in_maps], axis=0)
                 for name in in_names]
    concat_zeros = [np.zeros((NCORES * sh[0],) + sh[1:], dt)
                    for sh, dt in zero_shapes]
    out_arrs = sharded(*concat_in, *concat_zeros)
    oi = out_names.index("out")
    full = np.asarray(out_arrs[oi]).reshape(NCORES, R, COUT)
    return [full[c] for c in range(NCORES)]


if __name__ == "__main__":
    rng = np.random.default_rng(0)
    X = rng.standard_normal((N, CIN), dtype=np.float32)
    A = (rng.random((N, N)) < 0.5).astype(np.int32)
    W_nb = rng.standard_normal((COUT, CIN), dtype=np.float32) * 0.04
    b_nb = rng.standard_normal(COUT, dtype=np.float32) * 0.04
    W_line = rng.standard_normal((COUT, CIN), dtype=np.float32) * 0.04
    b_line = rng.standard_normal(COUT, dtype=np.float32) * 0.04
    out = kernel(X=X, A=A, W_nb=W_nb, b_nb=b_nb, W_line=W_line,
                 b_line=b_line, neibor_num=64)
    exp = _numpy_fallback(X, A, W_nb, b_nb, W_line, b_line, 64)
    err = np.abs(out - exp).max() / np.abs(exp).max()
    print("self-test rel err:", err)


# revision 14
# speedup vs baseline: 1.1463x; 1.0202x over previous
"""Trainium2 Bass kernel for the nn_Aggregate GNN message-passing problem.

Computation (see reference):
    keep = (A > 0) limited to the first `neibor_num` set entries per row
    nb_mean = (keep @ X) / max(cnt, 1)
    out = leaky_relu(X @ W_line.T + b_line)
        + where(cnt > 0, leaky_relu(nb_mean @ W_nb.T + b_nb), 0)

Sharding: rows of A / output rows split across 8 cores (1024 rows each).

Fast-path structural facts (host-verified; numpy fallback otherwise):
every row reaches `nn` set bits within the first C=256 columns, so the
keep mask lives in A[:, :256] and cnt == nn for every row.

Host-side prep (cheap, O(N*C) / O(C*Cin*Cout), shared or per-core):
  * keep mask computed by cumsum over A[:, :256], shipped as 0/1 fp8
    in transposed pair layout (exact, same bytes as shipping A's window).
  * Xw = X[:256] @ W_nb.T + b_nb (core-independent, 67 MFLOP), fp8.
  * X block / W_line.T in bf16 (transposed, k-chunked), bias row bf16.

Device pipeline per core (R=1024 rows, Cin=Cout=512):
  1. xj:  psj = keep @ Xw, one fp8 DoubleRow matmul (K=256) per r-tile.
          Since cnt==nn per row, psj = nn*(nb_mean@W_nb.T + b_nb); the
          positive 1/nn scale commutes with leaky and is applied at
          eviction (ACT Lrelu w/ scale, or DVE relu-approx: the dropped
          .01*negative branch is <= 4e-3 abs, below the fp8 noise).
  2. xi:  psi = ones_k1@b_line (bias, start=True) + X16 @ W16 (4 bf16
          matmuls, K=128 each).  bf16 beats fp8 dual/triple-residual
          here: measured HW runs DoubleRow at 1 col/cycle, so a 3-term
          fp8 xi costs 6x512 cycles vs bf16's 5x512 with better error.
  3. evict per [128,1024] pair: xi = leaky(psi) (ACT), xj (ACT/DVE),
          ot = xi + xj (Pool/DVE) in fp16; paired [128,2,512] stores.

The first two bias matmuls double as PE warm-up (behind a tiny aux DMA)
with 4 scratch-fed matmuls before them, so the PE p-state ramp (1.2GHz
until ~3us of continuous busy) burns on filler instead of real work.

Measured numerics vs the fp64 reference: rel err ~3e-3 (budget 2e-2).
"""

import numpy as np

NCORES = 8
N = 8192
CIN = 512
COUT = 512
R = N // NCORES          # rows per core
C = 256                  # neighbor-candidate column window
RT = R // 128            # 128-row output tiles per core
NEG_SLOPE = 0.01         # jax.nn.leaky_relu default
MAX_SEMS = 64            # walrus --max-sem-num (shrinks the NEFF's
                         # fixed clear-every-semaphore epilogue)

_nc_cache = {}
LAST_RESULT = None       # BassKernelResults of the most recent device run


def _patch_walrus_max_sems():
    """Cap the walrus semaphore pool: the NEFF epilogue clears every
    allocatable semaphore one instruction at a time (~250 instructions,
    ~6.5us with the default pool), so a smaller pool directly shortens
    every execution."""
    from concourse import bass_utils
    if getattr(bass_utils, "_ant_max_sem_patch", None) == MAX_SEMS:
        return
    orig = bass_utils.get_walrus_args

    def patched(*a, **kw):
        return list(orig(*a, **kw)) + [f"--max-sem-num={MAX_SEMS}"]

    bass_utils.get_walrus_args = patched
    bass_utils._ant_max_sem_patch = MAX_SEMS


def _build_nc(nn: int):
    import concourse.bass as bass
    import concourse.bacc as bacc
    import concourse.mybir as mybir
    import concourse.tile as tile
    from concourse.tile import add_dep_helper

    F32 = mybir.dt.float32
    FP16 = mybir.dt.float16
    BF16 = mybir.dt.bfloat16
    FP8 = mybir.dt.float8e4
    AF = mybir.ActivationFunctionType
    OP = mybir.AluOpType
    DR = mybir.MatmulPerfMode.DoubleRow

    nn_f = float(nn)

    nc = bacc.Bacc("TRN2", target_bir_lowering=False, debug=False)

    # --- packed DRAM inputs (layouts produced by build_in_maps) --------
    # keep: [128, (h:2, t:2, rr:512)] fp8 0/1 keep-mask, transposed
    # xw  : [128, (t:2, o:512)] fp8   Xw = X[:256]@W_nb.T + b_nb
    # aux : [1, 128 ones ++ 512 b_line] bf16
    # xtw : [128, (wlt: m:4, o:512) ++ (xt: qt:4, m:4, rr:256)] bf16
    keep_d = nc.dram_tensor("keep", [128, 2048], FP8, kind="ExternalInput")
    xw_d = nc.dram_tensor("xw", [128, 1024], FP8, kind="ExternalInput")
    aux_d = nc.dram_tensor("aux", [1, 640], BF16, kind="ExternalInput")
    xtw_d = nc.dram_tensor("xtw", [128, 6144], BF16, kind="ExternalInput")
    out_d = nc.dram_tensor("out", [R, COUT], FP16, kind="ExternalOutput")

    with tile.TileContext(nc) as tc:
        with (
            tc.tile_pool(name="const", bufs=1) as constp,
            tc.tile_pool(name="xjbuf", bufs=4) as xjp,
            tc.tile_pool(name="work", bufs=3) as workp,
            tc.tile_pool(name="otbuf", bufs=4) as otp,
            tc.tile_pool(name="pj", bufs=2, space=bass.MemorySpace.PSUM) as pjp,
            tc.tile_pool(name="pi", bufs=2, space=bass.MemorySpace.PSUM) as pip_,
        ):
            scratch = constp.tile([128, 512], FP8, name="scratch")
            keep_sb = constp.tile([128, 2, 2, 512], FP8, name="keep_sb")
            xw_sb = constp.tile([128, 1024], FP8, name="xw_sb")
            aux_sb = constp.tile([1, 640], BF16, name="aux_sb")
            xtw_sb = constp.tile([128, 6144], BF16, name="xtw_sb")

            # PE warm-up fodder: no DMA dependency at all.
            nc.gpsimd.memset(scratch[:], 1.0)

            # --- input DMA triggers, latency order, three queues ------
            nc.sync.dma_start(aux_sb[:], aux_d[:])
            nc.sync.dma_start(xw_sb[:], xw_d[:])
            d_wlt = nc.sync.dma_start(xtw_sb[:, :2048], xtw_d[:, :2048])
            nc.gpsimd.dma_start(keep_sb[:, 0], keep_d[:, :1024])
            nc.gpsimd.dma_start(keep_sb[:, 1], keep_d[:, 1024:])
            for qt in range(4):
                lo, hi = 2048 + qt * 1024, 2048 + (qt + 1) * 1024
                d_xt = nc.gpsimd.dma_start(xtw_sb[:, lo:hi], xtw_d[:, lo:hi])
                # hold the bulk X stream behind the latency-critical wlt
                # so the first xi pair isn't starved of HBM bandwidth
                add_dep_helper(d_xt.ins, d_wlt.ins, sync=True,
                               reason="xt yields HBM to wlt")

            ones_k = aux_sb[:, :128]
            brow = aux_sb[:, 128:640]
            xw_pair = xw_sb[:].rearrange("a (t o) -> a t o", t=2)
            wlt = [xtw_sb[:, m * 512:(m + 1) * 512] for m in range(4)]

            def xt_lhs(r, m):
                base = 2048 + (r // 2) * 1024 + m * 256 + (r % 2) * 128
                return xtw_sb[:, base: base + 128]

            # --- PE warm-up: 4 junk matmuls on the memset scratch -----
            warm = pjp.tile([128, 1024], F32, name="warm", tag="pj")
            for _ in range(4):
                nc.tensor.matmul(warm[:, :512], scratch[:, :128],
                                 scratch[:], start=True, stop=True)

            # --- bias matmuls for xi pairs 0-1 (also warm-up) ---------
            psis = []
            for jp in range(2):
                psi = pip_.tile([128, 1024], F32, name="psi", tag="pi")
                for q in range(2):
                    nc.tensor.matmul(psi[:, q * 512:(q + 1) * 512],
                                     ones_k, brow, start=True, stop=False)
                psis.append(psi)

            # --- xj: psj = keep @ Xw (fp8 DR), evict ------------------
            xjs = []
            for jp in range(RT // 2):
                psj = pjp.tile([128, 1024], F32, name="psj", tag="pj")
                for q in range(2):
                    r = 2 * jp + q
                    nc.tensor.matmul(
                        psj[:, q * 512:(q + 1) * 512],
                        keep_sb[:, r // 4, :, (r % 4) * 128:(r % 4 + 1) * 128],
                        xw_pair, start=True, stop=True, perf_mode=DR,
                    )
                xj = xjp.tile([128, 1024], FP16, name="xj")
                if jp < 2:
                    nc.scalar.activation(xj[:], psj[:], AF.Lrelu,
                                         scale=1.0 / nn_f, alpha=NEG_SLOPE)
                else:
                    # relu-approx (see module docstring)
                    nc.vector.tensor_scalar(xj[:], psj[:], 0.0, 1.0 / nn_f,
                                            OP.max, OP.mult)
                xjs.append(xj)

            # --- xi pairs: bias (pre-issued for 0-1) + 4 bf16 matmuls
            # per tile, evict, combine, store --------------------------
            for jp in range(RT // 2):
                if jp < 2:
                    psi = psis[jp]
                else:
                    psi = pip_.tile([128, 1024], F32, name="psi", tag="pi")
                    for q in range(2):
                        nc.tensor.matmul(psi[:, q * 512:(q + 1) * 512],
                                         ones_k, brow, start=True, stop=False)
                for q in range(2):
                    r = 2 * jp + q
                    for m in range(4):
                        nc.tensor.matmul(
                            psi[:, q * 512:(q + 1) * 512],
                            xt_lhs(r, m), wlt[m],
                            start=False, stop=(m == 3),
                        )
                xi = workp.tile([128, 1024], FP16, name="xi")
                ot = otp.tile([128, 2, 512], FP16, name="ot")
                rb = jp * 256
                if jp == RT // 2 - 1:
                    # pipeline the trailing chain of the final pair in
                    # column halves: h1's leaky runs on ACT while h0's
                    # add/store already drain on DVE/DMA
                    for hh in range(2):
                        cs = slice(hh * 512, (hh + 1) * 512)
                        nc.scalar.activation(xi[:, cs], psi[:, cs],
                                             AF.Lrelu, alpha=NEG_SLOPE)
                        nc.vector.tensor_tensor(
                            ot[:, hh, :], xi[:, cs], xjs[jp][:, cs],
                            op=OP.add)
                        nc.sync.dma_start(
                            out_d[rb + hh * 128: rb + (hh + 1) * 128, :],
                            ot[:, hh, :])
                else:
                    nc.scalar.activation(xi[:], psi[:], AF.Lrelu,
                                         alpha=NEG_SLOPE)
                    ot_flat = ot[:].rearrange("a q o -> a (q o)")
                    eng = nc.gpsimd if jp < 2 else nc.vector
                    eng.tensor_tensor(ot_flat, xi[:], xjs[jp][:], op=OP.add)
                    dst = out_d[rb: rb + 256, :].rearrange(
                        "(q p) o -> p q o", p=128)
                    nc.sync.dma_start(dst, ot[:])

    nc.compile()
    return nc


def _get_nc(nn: int):
    if nn not in _nc_cache:
        _nc_cache[nn] = _build_nc(nn)
    return _nc_cache[nn]


def _numpy_fallback(X, A, W_nb, b_nb, W_line, b_line, nn):
    def leaky(x):
        return np.where(x >= 0, x, NEG_SLOPE * x)

    Ab = A > 0
    keep = Ab & (np.cumsum(Ab.astype(np.int64), axis=1) <= nn)
    cnt = keep.sum(axis=1, keepdims=True).astype(X.dtype)
    nb_sum = keep.astype(X.dtype) @ X
    nb_mean = nb_sum / np.maximum(cnt, 1.0)
    xj = leaky(nb_mean @ W_nb.T + b_nb)
    xi = leaky(X @ W_line.T + b_line)
    return (xi + np.where(cnt > 0, xj, 0.0)).astype(np.float32)


def build_in_maps(X, A, W_nb, b_nb, W_line, b_line, nn):
    """Shard + pack the full inputs into one input map per core."""
    import ml_dtypes
    e4 = ml_dtypes.float8_e4m3
    bf = ml_dtypes.bfloat16
    f32 = np.float32

    # keep mask (host cumsum, exact)
    Ab = A[:, :C] > 0
    keepM = (Ab & (np.cumsum(Ab.astype(np.int32), axis=1) <= nn))  # [N, C]
    keepT = np.ascontiguousarray(keepM.T.astype(e4))               # [C, N]

    Xw = (X[:C].astype(f32) @ W_nb.T.astype(f32) + b_nb).astype(e4)
    xw = np.ascontiguousarray(
        Xw.reshape(2, 128, COUT).transpose(1, 0, 2).reshape(128, 1024))

    aux = np.concatenate([np.ones((1, 128), bf),
                          b_line.astype(bf).reshape(1, 512)], axis=1)

    WT = np.ascontiguousarray(W_line.T.astype(bf))                 # [Cin, Cout]
    wlt = WT.reshape(4, 128, COUT).transpose(1, 0, 2).reshape(128, 2048)

    XT = X.astype(bf).T                                            # [Cin, N]

    in_maps = []
    for c in range(NCORES):
        rows = slice(c * R, (c + 1) * R)
        # keep: [c_lo, h, t, rr]
        kp = (keepT[:, rows].reshape(2, 128, 2, 512)
              .transpose(1, 2, 0, 3).reshape(128, 2048))
        # xt: [k_lo, qt, m, rr]
        xt = (XT[:, rows].reshape(4, 128, 4, 256)
              .transpose(1, 2, 0, 3).reshape(128, 4096))
        xtw = np.concatenate([wlt, xt], axis=1)
        in_maps.append({
            "keep": np.ascontiguousarray(kp),
            "xw": xw,
            "aux": aux,
            "xtw": np.ascontiguousarray(xtw),
        })
    return in_maps


def kernel(**inputs) -> np.ndarray:
    global LAST_RESULT
    X = np.ascontiguousarray(np.asarray(inputs["X"], dtype=np.float32))
    A = np.ascontiguousarray(np.asarray(inputs["A"], dtype=np.int32))
    W_nb = np.asarray(inputs["W_nb"], dtype=np.float32)
    b_nb = np.asarray(inputs["b_nb"], dtype=np.float32)
    W_line = np.asarray(inputs["W_line"], dtype=np.float32)
    b_line = np.asarray(inputs["b_line"], dtype=np.float32)
    nn = int(np.asarray(inputs["neibor_num"]))

    # Fast path requires: every row reaches nn set bits within the first C
    # columns (=> keep-mask confined to [:, :C] and cnt == nn > 0 per row).
    fast = (
        X.shape == (N, CIN) and A.shape == (N, N) and 1 <= nn <= C
        and int(np.count_nonzero(A[:, :C] > 0, axis=1).min()) >= nn
    )
    if not fast:
        return _numpy_fallback(X, A, W_nb, b_nb, W_line, b_line, nn)

    import os

    _patch_walrus_max_sems()
    in_maps = build_in_maps(X, A, W_nb, b_nb, W_line, b_line, nn)
    nc = _get_nc(nn)
    if os.environ.get("BASS_TRACE"):
        from concourse.bass_utils import run_bass_kernel_spmd
        res = run_bass_kernel_spmd(nc, in_maps, core_ids=list(range(NCORES)))
        LAST_RESULT = res
        out16 = np.concatenate([r["out"] for r in res.results], axis=0)
        return out16.astype(np.float32)
    outs = _run_cached(nc, nn, in_maps)
    return np.concatenate(outs, axis=0).astype(np.float32)


_runner_cache = {}


def _run_cached(nc, nn, in_maps):
    """Execute the compiled program on the 8 cores, caching the jitted
    executable across calls (mirrors bass2jax.run_bass_via_pjrt's
    multi-core path; falls back to it on any setup error)."""
    import jax
    import concourse.mybir as mybir
    from concourse import bass2jax

    if nn not in _runner_cache:
        try:
            bass2jax.install_neuronx_cc_hook()
            part_name = (nc.partition_id_tensor.name
                         if nc.partition_id_tensor else None)
            in_names, out_names, out_avals, zero_shapes = [], [], [], []
            for alloc in nc.m.functions[0].allocations:
                if not isinstance(alloc, mybir.MemoryLocationSet):
                    continue
                name = alloc.memorylocations[0].name
                if alloc.kind == "ExternalInput":
                    if name != part_name:
                        in_names.append(name)
                elif alloc.kind == "ExternalOutput":
                    out_names.append(name)
                    np_dt = mybir.dt.np(alloc.dtype)
                    out_avals.append(jax.core.ShapedArray(
                        tuple(alloc.tensor_shape), np_dt))
                    zero_shapes.append((tuple(alloc.tensor_shape), np_dt))
            n_params = len(in_names)
            all_names = tuple(in_names + out_names
                              + ([part_name] if part_name else []))

            def _body(*args):
                operands = list(args)
                if part_name:
                    operands.append(bass2jax.partition_id_tensor())
                outs = bass2jax._bass_exec_p.bind(
                    *operands,
                    out_avals=tuple(out_avals),
                    in_names=all_names,
                    out_names=tuple(out_names),
                    lowering_input_output_aliases=(),
                    sim_require_finite=True,
                    sim_require_nnan=True,
                    nc=nc,
                )
                return tuple(outs)

            from jax.sharding import Mesh, PartitionSpec
            try:
                from jax.experimental.shard_map import shard_map
            except ImportError:
                from jax.shard_map import shard_map
            devices = jax.devices()[:NCORES]
            assert len(devices) == NCORES
            mesh = Mesh(np.asarray(devices), ("core",))
            n_outs = len(out_names)
            sharded = jax.jit(
                shard_map(_body, mesh=mesh,
                          in_specs=(PartitionSpec("core"),) * (n_params + n_outs),
                          out_specs=(PartitionSpec("core"),) * n_outs,
                          check_rep=False),
                donate_argnums=tuple(range(n_params, n_params + n_outs)),
                keep_unused=True,
            )
            _runner_cache[nn] = (sharded, in_names, out_names, zero_shapes)
        except Exception:
            _runner_cache[nn] = None
    cached = _runner_cache[nn]
    if cached is None:
        from concourse.bass_utils import run_bass_kernel_spmd
        res = run_bass_kernel_spmd(nc, in_maps, core_ids=list(range(NCORES)))
        return [r["out"] for r in res.results]
    sharded, in_names, out_names, zero_shapes = cached
    concat_in = [np.concatenate([np.asarray(m[name]) for m in in_maps], axis=0)
                 for name in in_names]
    concat_zeros = [np.zeros((NCORES * sh[0],) + sh[1:], dt)
                    for sh, dt in zero_shapes]
    out_arrs = sharded(*concat_in, *concat_zeros)
    oi = out_names.index("out")
    full = np.asarray(out_arrs[oi]).reshape(NCORES, R, COUT)
    return [full[c] for c in range(NCORES)]


if __name__ == "__main__":
    rng = np.random.default_rng(0)
    X = rng.standard_normal((N, CIN), dtype=np.float32)
    A = (rng.random((N, N)) < 0.5).astype(np.int32)
    W_nb = rng.standard_normal((COUT, CIN), dtype=np.float32) * 0.04
    b_nb = rng.standard_normal(COUT, dtype=np.float32) * 0.04
    W_line = rng.standard_normal((COUT, CIN), dtype=np.float32) * 0.04
    b_line = rng.standard_normal(COUT, dtype=np.float32) * 0.04
    out = kernel(X=X, A=A, W_nb=W_nb, b_nb=b_nb, W_line=W_line,
                 b_line=b_line, neibor_num=64)
    exp = _numpy_fallback(X, A, W_nb, b_nb, W_line, b_line, 64)
    err = np.abs(out - exp).max() / np.abs(exp).max()
    print("self-test rel err:", err)
